# revision 1
# baseline (speedup 1.0000x reference)
"""Trainium2 Bass kernel for nn_MHLMachine (sparse relative-position attention).

Self-contained: kernel(**inputs) takes the FULL inputs (x, generator, Wq, Wv,
Wproj), shards batch-parallel across 8 NeuronCores via bass/PJRT (axon), and
returns the full (8, 1024, 1024) float32 output.

Per-core program (one batch element per core):
  qT = Wq @ x.T (fp32r GEMM), v = x @ Wv.T (fp32r GEMM, bf16 out)
  per head: w_raw window GEMM (fp32r) -> bf16 -> DRAM scratch ->
    Toeplitz skew via diagonal-AP DMA -> cumsum via triangular-ones
    matmuls with PSUM-resident running state -> relu+tril mask (ACT +
    gpsimd affine_select) -> row-sum normalize -> PE transposes ->
    AV (bf16, N=512) -> attnT
  out = attnT.T-slices @ Wproj.T (bf16 GEMM) -> fp32
"""
import contextlib

import numpy as np
import ml_dtypes
import jax
from jax.sharding import Mesh, PartitionSpec
from jax.experimental.shard_map import shard_map

import concourse.bass as bass
import concourse.mybir as mybir
import concourse.tile as tile
from concourse.bass import AP
from concourse import bass2jax
from concourse.bass2jax import _bass_exec_p, install_neuronx_cc_hook, partition_id_tensor

F32 = mybir.dt.float32
F32R = mybir.dt.float32r
BF16 = mybir.dt.bfloat16
AF = mybir.ActivationFunctionType


L = 1024
D = 1024
H = 4
DH = 256          # head dim
NT = 8            # 128-tiles per 1024
GW = 2047         # generator width
WW = 1152         # w_raw window width per l-block


def win_start(k):
    return max(0, min(896 - 128 * k, GW - WW))  # width WW covers needed [896-128k, 2046-128k]


def build(nc, reps=1):
    # ---------------- I/O ----------------
    xT32 = nc.dram_tensor("xT32", [NT, 128, L], F32, kind="ExternalInput")     # x.T (k,l)
    wqT = nc.dram_tensor("wqT", [NT, 128, D], F32, kind="ExternalInput")       # Wq.T (k,e)
    wvT = nc.dram_tensor("wvT", [NT, 128, D], F32, kind="ExternalInput")       # Wv.T (k,e)
    gen_d = nc.dram_tensor("gen", [H, 2, 128, GW], F32, kind="ExternalInput")  # (h, ddt, dd, j)
    wpT = nc.dram_tensor("wpT", [NT, 128, D], BF16, kind="ExternalInput")      # Wproj.T (e,f)
    out_d = nc.dram_tensor("out", [NT, 128, D], F32, kind="ExternalOutput")    # (l, f)

    with tile.TileContext(nc) as tc:
        with contextlib.ExitStack() as es:
            pconst = es.enter_context(tc.tile_pool(name="const", bufs=1))
            pqt = es.enter_context(tc.tile_pool(name="qt", bufs=1))
            pv = es.enter_context(tc.tile_pool(name="vv", bufs=1))
            pat = es.enter_context(tc.tile_pool(name="at", bufs=1))
            p3w = es.enter_context(tc.tile_pool(name="p3w", bufs=1))
            p3o = es.enter_context(tc.tile_pool(name="p3o", bufs=2))
            p3ps = es.enter_context(tc.tile_pool(name="p3ps", bufs=1, space="PSUM"))
            # ---------------- constants ----------------
            # affine_select: out = predicate ? in_ : fill
            ident = pconst.tile([128, 128], BF16, tag="ident")
            nc.gpsimd.memset(ident[:], 1.0)
            nc.gpsimd.affine_select(   # keep where p - c == 0
                out=ident[:], in_=ident[:], compare_op=mybir.AluOpType.is_equal,
                fill=0.0, base=0, pattern=[[-1, 128]], channel_multiplier=1)
            # U[l', l] = 1 iff l' <= l  (upper triangular incl diag)
            u_tri = pconst.tile([128, 128], BF16, tag="u_tri")
            nc.gpsimd.memset(u_tri[:], 1.0)
            nc.gpsimd.affine_select(   # keep where l - l' >= 0
                out=u_tri[:], in_=u_tri[:], compare_op=mybir.AluOpType.is_ge,
                fill=0.0, base=0, pattern=[[1, 128]], channel_multiplier=-1)
            # L'[l', l] = 1 iff l' > l (strictly lower)
            lp_tri = pconst.tile([128, 128], BF16, tag="lp_tri")
            nc.gpsimd.memset(lp_tri[:], 1.0)
            nc.gpsimd.affine_select(   # keep where l' - l > 0
                out=lp_tri[:], in_=lp_tri[:], compare_op=mybir.AluOpType.is_gt,
                fill=0.0, base=0, pattern=[[-1, 128]], channel_multiplier=1)
            ones_t = pconst.tile([128, 128], BF16, tag="ones_t")
            nc.gpsimd.memset(ones_t[:], 1.0)

            qT = [pqt.tile([128, L], F32R, tag=f"qT{i}", name=f"qT{i}") for i in range(NT)]
            v_sb = [pv.tile([128, D], BF16, tag=f"v{i}", name=f"v{i}") for i in range(NT)]
            attnT = [pat.tile([128, L], BF16, tag=f"at{i}", name=f"at{i}") for i in range(NT)]

            for _rep in range(reps):
                if _rep > 0:
                    tc.strict_bb_all_engine_barrier()
                # ---------------- phase 1: q & v GEMMs ----------------
                with contextlib.ExitStack() as es1:
                    p1w = es1.enter_context(tc.tile_pool(name="p1w", bufs=1))
                    p1ps = es1.enter_context(tc.tile_pool(name="p1ps", bufs=3, space="PSUM"))
                    wp_w = p3w.tile([128, NT, D], BF16, tag="wpw")
                    nc.sync.dma_start(wp_w[:], wpT.rearrange("e p f -> p e f"))
                    xt = p1w.tile([128, NT, L], F32R, tag="xt")
                    wq = p1w.tile([128, NT, D], F32R, tag="wq")
                    wv = p1w.tile([128, NT, D], F32R, tag="wv")
                    for k in range(NT):
                        nc.gpsimd.dma_start(xt[:, k, :], xT32[k])
                        nc.gpsimd.dma_start(wq[:, k, :], wqT[k])
                    for k in range(NT):
                        nc.gpsimd.dma_start(wv[:, k, :], wvT[k])

                    # qT[e,l]: lhsT=wq[k][:,e-tile], rhs=xt[k][:,l-chunk]
                    for et in range(NT):
                        for lc in range(2):
                            ps = p1ps.tile([128, 512], F32, tag="ps_q")
                            for k in range(NT):
                                nc.tensor.matmul(
                                    ps[:], wq[:, k, et * 128:(et + 1) * 128],
                                    xt[:, k, lc * 512:(lc + 1) * 512],
                                    start=(k == 0), stop=(k == NT - 1))
                            nc.vector.tensor_copy(qT[et][:, lc * 512:(lc + 1) * 512], ps[:])
                    # v[l,e]: lhsT=xt[k][:,l-tile], rhs=wv[k][:,e-chunk]
                    for lt in range(NT):
                        for ec in range(2):
                            ps = p1ps.tile([128, 512], F32, tag="ps_v")
                            for k in range(NT):
                                nc.tensor.matmul(
                                    ps[:], xt[:, k, lt * 128:(lt + 1) * 128],
                                    wv[:, k, ec * 512:(ec + 1) * 512],
                                    start=(k == 0), stop=(k == NT - 1))
                            nc.scalar.activation(v_sb[lt][:, ec * 512:(ec + 1) * 512], ps[:], AF.Copy)

                # phase-2 pools reuse phase-1 SBUF space; fence so their writes
                # cannot race phase-1 readers (CoreSim race detector caught this)
                tc.strict_bb_all_engine_barrier()
                # ---------------- phase 2: heads ----------------
                with contextlib.ExitStack() as es2:
                    pgen = es2.enter_context(tc.tile_pool(name="gen", bufs=3))
                    pW = es2.enter_context(tc.tile_pool(name="wt", bufs=2))
                    pstg = es2.enter_context(tc.tile_pool(name="stg", bufs=3))
                    pw2 = es2.enter_context(tc.tile_pool(name="w2", bufs=4))
                    pwp = es2.enter_context(tc.tile_pool(name="wp", bufs=2))
                    pz = es2.enter_context(tc.tile_pool(name="zz", bufs=8))
                    pscr = es2.enter_context(tc.tile_pool(name="scr", bufs=8, space="DRAM"))
                    pcps = es2.enter_context(tc.tile_pool(name="cps", bufs=1, space="PSUM"))
                    pwrps = es2.enter_context(tc.tile_pool(name="wrps", bufs=1, space="PSUM"))
                    ptps = es2.enter_context(tc.tile_pool(name="tps", bufs=1, space="PSUM"))
                    pavps = es2.enter_context(tc.tile_pool(name="avps", bufs=1, space="PSUM"))
                    for h in range(H):
                        g = pgen.tile([128, 2, GW], F32R, tag="gen")
                        nc.gpsimd.dma_start(g[:], gen_d[h].rearrange("t p j -> p t j"))
                        W = pW.tile([128, NT * L], BF16, tag="W")   # w'T packed: [:, j*1024 + l]
                        # zero the always-invalid regions AV will read (l < 128j)
                        for j in (1, 2, 3):
                            nc.gpsimd.memset(W[:, j * L: j * L + 128 * j], 0.0)
                        for j in (5, 6, 7):
                            nc.gpsimd.memset(W[:, j * L + 512: j * L + 128 * j], 0.0)

                        C = pcps.tile([128, L], F32, tag="C")       # cumsum accumulator (2 banks)

                        for k in range(NT):
                            s = win_start(k)
                            # --- w_raw window GEMM (3 x 384 chunks, own banks) ---
                            stg = pstg.tile([128, WW], BF16, tag="stg")
                            for ci in range(3):
                                c0 = ci * 384
                                wr = pwrps.tile([128, 384], F32, tag=f"wr{ci}", name=f"wr{ci}")
                                for dd in range(2):
                                    nc.tensor.matmul(
                                        wr[:],
                                        qT[2 * h + dd][:, k * 128:(k + 1) * 128],
                                        g[:, dd, s + c0: s + c0 + 384],
                                        start=(dd == 0), stop=(dd == 1))
                                if (k + ci) % 2 == 0:
                                    nc.vector.tensor_copy(stg[:, c0:c0 + 384], wr[:])
                                else:
                                    nc.scalar.activation(stg[:, c0:c0 + 384], wr[:], AF.Copy)
                            scr = pscr.tile([128, WW], BF16, tag="scr")
                            nc.sync.dma_start(scr[:], stg[:])
                            # --- skew load: w2[p, i] = scr[p, off0 - p + i] ---
                            off0 = 1023 - 128 * k - s
                            w2 = pw2.tile([128, L], BF16, tag="w2")
                            diag = AP(tensor=scr.tensor, offset=scr.offset + off0,
                                      ap=[[WW - 1, 128], [1, L]])
                            nc.sync.dma_start(w2[:], diag)
                            # --- cumsum: C += U @ w2; chunk 1 unread until k=4 -> use F=U+L' ---
                            nc.tensor.matmul(C[:, 0:512], u_tri[:], w2[:, 0:512],
                                             start=(k == 0), stop=(k == NT - 1))
                            c1_m = ones_t if k <= 3 else u_tri
                            nc.tensor.matmul(C[:, 512:1024], c1_m[:], w2[:, 512:1024],
                                             start=(k == 0), stop=(k == NT - 1))
                            # --- consume: relu copy, mask, rowsum, normalize ---
                            ncols = 128 * (k + 1)
                            wp_t = pwp.tile([128, L], BF16, tag="wp")
                            nc.scalar.activation(wp_t[:, :ncols], C[:, :ncols], AF.Relu)
                            nc.gpsimd.affine_select(
                                out=wp_t[:, k * 128:ncols], in_=wp_t[:, k * 128:ncols],
                                compare_op=mybir.AluOpType.is_ge,
                                fill=0.0, base=0, pattern=[[-1, 128]], channel_multiplier=1)
                            zr = pz.tile([128, 1], F32, tag="zr")
                            nc.vector.tensor_reduce(zr[:], wp_t[:, :ncols],
                                                    axis=mybir.AxisListType.X,
                                                    op=mybir.AluOpType.add)
                            nc.vector.tensor_scalar_add(zr[:], zr[:], 1e-8)
                            nc.vector.reciprocal(zr[:], zr[:])
                            nc.vector.tensor_scalar_mul(wp_t[:, :ncols], wp_t[:, :ncols], zr[:])
                            # --- release C: += L' @ w2 (chunk 1 already has F for k<=3) ---
                            if k < NT - 1:
                                nc.tensor.matmul(C[:, 0:512], lp_tri[:], w2[:, 0:512],
                                                 start=False, stop=False)
                                if k > 3:
                                    nc.tensor.matmul(C[:, 512:1024], lp_tri[:], w2[:, 512:1024],
                                                     start=False, stop=False)
                            # --- transposes: w'T[j][:, k-block] for j <= k ---
                            for jp in range(0, k + 1, 4):
                                nj = min(4, k + 1 - jp)
                                tp = ptps.tile([128, 512], BF16, tag="tp")
                                for ji in range(nj):
                                    j = jp + ji
                                    nc.tensor.transpose(
                                        tp[:, ji * 128:(ji + 1) * 128],
                                        wp_t[:, j * 128:(j + 1) * 128], ident[:])
                                dst = W.rearrange("p (j l) -> p j l", j=NT)[
                                    :, jp:jp + nj, k * 128:(k + 1) * 128]
                                srcv = tp[:, :nj * 128].rearrange("p (j l) -> p j l", j=nj)
                                nc.scalar.activation(dst, srcv, AF.Copy)
                            # --- AV for completed l-chunk ---
                            if k == 3 or k == 7:
                                c = k // 4
                                # sub-chunks (start, width, jmax): skip all-zero W regions
                                subs = ([(0, 512, 3)] if c == 0
                                        else [(512, 256, 5), (768, 256, 7)])
                                for m in range(2):
                                    av = pavps.tile([128, 512], F32, tag="av", name="av")
                                    for s0, sw, jmax in subs:
                                        for j in range(jmax + 1):
                                            nc.tensor.matmul(
                                                av[:, s0 - c * 512: s0 - c * 512 + sw],
                                                v_sb[j][:, h * DH + m * 128: h * DH + (m + 1) * 128],
                                                W[:, j * L + s0: j * L + s0 + sw],
                                                start=(j == 0), stop=(j == jmax))
                                    nc.scalar.activation(
                                        attnT[2 * h + m][:, c * 512:(c + 1) * 512],
                                        av[:], AF.Copy)

                # ---------------- phase 3: projection ----------------
                for lt in range(NT):
                    osb = p3o.tile([128, D], F32, tag="osb")
                    for fc in range(2):
                        ps = p3ps.tile([128, 512], F32, tag="ps_o")
                        for e in range(NT):
                            nc.tensor.matmul(
                                ps[:], attnT[e][:, lt * 128:(lt + 1) * 128],
                                wp_w[:, e, fc * 512:(fc + 1) * 512],
                                start=(e == 0), stop=(e == NT - 1))
                        if fc == 0:
                            nc.vector.tensor_copy(osb[:, fc * 512:(fc + 1) * 512], ps[:])
                        else:
                            nc.scalar.activation(osb[:, fc * 512:(fc + 1) * 512], ps[:], AF.Copy)
                    nc.sync.dma_start(out_d[lt], osb[:])


    return nc


def make_in_maps(x, generator, Wq, Wv, Wproj):
    """Full inputs -> list of 8 per-core input dicts."""
    B = x.shape[0]
    bf16 = ml_dtypes.bfloat16
    wqT = np.ascontiguousarray(Wq.T.astype(np.float32)).reshape(NT, 128, D)
    wvT = np.ascontiguousarray(Wv.T.astype(np.float32)).reshape(NT, 128, D)
    wpT = np.ascontiguousarray(Wproj.T.astype(bf16)).reshape(NT, 128, D)
    gen = np.ascontiguousarray(generator.astype(np.float32)).reshape(H, 2, 128, GW)
    maps = []
    for b in range(B):
        xT = np.ascontiguousarray(x[b].T.astype(np.float32)).reshape(NT, 128, L)
        maps.append({"xT32": xT, "wqT": wqT, "wvT": wvT, "gen": gen, "wpT": wpT})
    return maps


# ---------------------------------------------------------------------------
# post-pass: ISA instructions carry one wait slot; split extras onto NoOps


def split_excess_waits(nc, keep=1):
    n_split = 0
    for fn in nc.m.functions:
        for blk in fn.blocks:
            insts = list(blk.instructions)
            out = []
            changed = False
            for inst in insts:
                si = inst.sync_info
                waits = list(si.on_wait) if si is not None and si.on_wait else []
                if len(waits) > keep:
                    for j, w in enumerate(waits[:-keep]):
                        nop = mybir.InstNoOp(name=f"{inst.name}-ws{j}", ins=[], outs=[])
                        nop.engine = inst.engine
                        nop.sync_info = mybir.SyncInfo(on_wait=[w], on_update=[])
                        out.append(nop)
                        nc.register_instruction(nop, overwrite=True)
                    inst.sync_info = mybir.SyncInfo(
                        on_wait=waits[-keep:],
                        on_update=list(si.on_update) if si.on_update else [],
                    )
                    changed = True
                    n_split += 1
                out.append(inst)
            if changed:
                try:
                    blk.instructions = out
                except Exception:
                    blk.instructions.clear()
                    blk.instructions.extend(out)
    return n_split

# ---------------------------------------------------------------------------
class _SpmdRunner:
    def __init__(self, nc, n_cores):
        install_neuronx_cc_hook()
        self.nc = nc
        self.n_cores = n_cores
        partition_name = nc.partition_id_tensor.name if nc.partition_id_tensor else None
        in_names, out_names, out_avals = [], [], []
        for alloc in nc.m.functions[0].allocations:
            if not isinstance(alloc, mybir.MemoryLocationSet):
                continue
            name = alloc.memorylocations[0].name
            if alloc.kind == "ExternalInput":
                if name != partition_name:
                    in_names.append(name)
            elif alloc.kind == "ExternalOutput":
                out_names.append(name)
                out_avals.append(jax.core.ShapedArray(
                    tuple(alloc.tensor_shape), mybir.dt.np(alloc.dtype)))
        self.in_names, self.out_names, self.out_avals = in_names, out_names, out_avals
        n_params, n_outs = len(in_names), len(out_avals)
        all_in = in_names + out_names + ([partition_name] if partition_name else [])

        def _body(*args):
            operands = list(args)
            if partition_name is not None:
                operands.append(partition_id_tensor())
            return tuple(_bass_exec_p.bind(
                *operands, out_avals=tuple(out_avals), in_names=tuple(all_in),
                out_names=tuple(out_names), lowering_input_output_aliases=(),
                sim_require_finite=False, sim_require_nnan=False, nc=nc))

        devices = jax.devices()[:n_cores]
        assert len(devices) == n_cores, f"need {n_cores} neuron cores, have {len(jax.devices())}"
        self.mesh = Mesh(np.asarray(devices), ("core",))
        in_specs = (PartitionSpec("core"),) * (n_params + n_outs)
        out_specs = (PartitionSpec("core"),) * n_outs
        self.fn = jax.jit(shard_map(_body, mesh=self.mesh, in_specs=in_specs,
                                    out_specs=out_specs, check_rep=False),
                          keep_unused=True)
        self._dev_args = None

    def set_inputs(self, in_maps):
        n = self.n_cores
        args = [np.concatenate([np.asarray(in_maps[c][nm]) for c in range(n)], axis=0)
                for nm in self.in_names]
        for av in self.out_avals:
            args.append(np.zeros((n * av.shape[0], *av.shape[1:]), av.dtype))
        sharding = jax.sharding.NamedSharding(self.mesh, PartitionSpec("core"))
        self._dev_args = [jax.device_put(a, sharding) for a in args]

    def run(self):
        outs = self.fn(*self._dev_args)
        jax.block_until_ready(outs)
        return outs

    def outputs_np(self, outs):
        n = self.n_cores
        return [{nm: np.asarray(outs[i]).reshape(n, *self.out_avals[i].shape)[c]
                 for i, nm in enumerate(self.out_names)} for c in range(n)]


_CACHE = {}


def _get_runner(reps=1):
    if reps not in _CACHE:
        nc = bass.Bass(target_bir_lowering=False)
        build(nc, reps=reps)
        split_excess_waits(nc)
        _CACHE[reps] = _SpmdRunner(nc, 8)
    return _CACHE[reps]


def kernel(x, generator, Wq, Wv, Wproj):
    x = np.asarray(x); generator = np.asarray(generator)
    in_maps = make_in_maps(x, generator, np.asarray(Wq), np.asarray(Wv), np.asarray(Wproj))
    runner = _get_runner()
    runner.set_inputs(in_maps)
    outs = runner.outputs_np(runner.run())
    return np.stack([outs[b]["out"].reshape(L, D) for b in range(x.shape[0])]).astype(np.float32)



# revision 19
# speedup vs baseline: 1.7935x; 1.7935x over previous
"""Trainium2 Bass kernel for nn_MHLMachine (sparse relative-position attention).

Self-contained: kernel(**inputs) takes the FULL inputs (x, generator, Wq, Wv,
Wproj), shards batch-parallel across 8 NeuronCores via bass/PJRT (axon), and
returns the full (8, 1024, 1024) float32 output.

Per-core program (one batch element per core), all-bf16 data path:
  qT = Wq @ x.T, v = x @ Wv.T (bf16 GEMMs, f32 PSUM)
  per head, per 128-row block k:
    w_raw window GEMM -> bf16 stg -> DRAM scratch -> Toeplitz skew via
    diagonal-AP DMA -> block prefix P = U @ w2 + ones x T_{k-1} (PSUM),
    carry row T_k = P[127,:] (fp16) -> relu+rowsum via activation accum_out
    (diag block masked by tril multiply) -> transpose-with-normalize via
    matmul against D = ident * (1/rowsum) -> AV (bf16)
  out = attnT.T-slices @ Wproj.T -> fp32
The head loop is split into two k-passes (k<4, k>=4) so the l<512 half of
the projection overlaps the second pass; q/v GEMM jobs are interleaved into
head-loop slots so the PE never idles waiting for phase boundaries.
"""
import contextlib

import numpy as np
import ml_dtypes
import jax
from jax.sharding import Mesh, PartitionSpec
from jax.experimental.shard_map import shard_map

import concourse.bass as bass
import concourse.mybir as mybir
import concourse.tile as tile
from concourse.bass import AP
from concourse import bass2jax
from concourse.bass2jax import _bass_exec_p, install_neuronx_cc_hook, partition_id_tensor

F32 = mybir.dt.float32
F16 = mybir.dt.float16
BF16 = mybir.dt.bfloat16
AF = mybir.ActivationFunctionType
ALU = mybir.AluOpType
AX = mybir.AxisListType


L = 1024
D = 1024
H = 4
DH = 256          # head dim
NT = 8            # 128-tiles per 1024
GW = 2047         # generator width
WW = 1152         # w_raw window width per l-block


def win_start(k):
    return max(0, min(896 - 128 * k, GW - WW))  # width WW covers needed [896-128k, 2046-128k]


def build(nc, reps=1):
    # ---------------- I/O ----------------
    xT_d = nc.dram_tensor("xT", [NT, 128, L], F16, kind="ExternalInput")      # x.T (k,l)
    wqT = nc.dram_tensor("wqT", [NT, 128, D], F16, kind="ExternalInput")      # Wq.T (k,e)
    wvT = nc.dram_tensor("wvT", [NT, 128, D], F16, kind="ExternalInput")      # Wv.T (k,e)
    gen_d = nc.dram_tensor("gen", [H, 2, 128, GW], F16, kind="ExternalInput") # (h, ddt, dd, j)
    wpT = nc.dram_tensor("wpT", [NT, 128, D], BF16, kind="ExternalInput")      # Wproj.T (e,f)
    out_d = nc.dram_tensor("out", [NT, 128, D], F32, kind="ExternalOutput")    # (l, f)

    with tile.TileContext(nc) as tc:
        with contextlib.ExitStack() as es:
            pconst = es.enter_context(tc.tile_pool(name="const", bufs=1))
            pin = es.enter_context(tc.tile_pool(name="pin", bufs=1))
            pqt = es.enter_context(tc.tile_pool(name="qt", bufs=1))
            pv = es.enter_context(tc.tile_pool(name="vv", bufs=1))
            pat = es.enter_context(tc.tile_pool(name="at", bufs=1))
            pgen = es.enter_context(tc.tile_pool(name="gen", bufs=3))
            pW = es.enter_context(tc.tile_pool(name="ww", bufs=2))
            pstg = es.enter_context(tc.tile_pool(name="stg", bufs=3))
            pw2 = es.enter_context(tc.tile_pool(name="w2", bufs=4))
            pwp = es.enter_context(tc.tile_pool(name="wp", bufs=2))
            pz = es.enter_context(tc.tile_pool(name="zz", bufs=4))
            pD = es.enter_context(tc.tile_pool(name="dd", bufs=2))
            po = es.enter_context(tc.tile_pool(name="oo", bufs=2))
            pscr = es.enter_context(tc.tile_pool(name="scr", bufs=8, space="DRAM"))
            psA = es.enter_context(tc.tile_pool(name="psA", bufs=3, space="PSUM"))
            psC = es.enter_context(tc.tile_pool(name="psC", bufs=1, space="PSUM"))
            psT = es.enter_context(tc.tile_pool(name="psT", bufs=2, space="PSUM"))
            psV = es.enter_context(tc.tile_pool(name="psV", bufs=1, space="PSUM"))

            # ---------------- constants ----------------
            # affine_select: out = predicate ? in_ : fill
            ident = pconst.tile([128, 128], BF16, tag="ident")
            nc.gpsimd.memset(ident[:], 1.0)
            nc.gpsimd.affine_select(   # keep where p - c == 0
                out=ident[:], in_=ident[:], compare_op=ALU.is_equal,
                fill=0.0, base=0, pattern=[[-1, 128]], channel_multiplier=1)
            # U[l', l] = 1 iff l' <= l  (upper triangular incl diag)
            u_tri = pconst.tile([128, 128], F16, tag="u_tri")
            nc.gpsimd.memset(u_tri[:], 1.0)
            nc.gpsimd.affine_select(   # keep where l - l' >= 0
                out=u_tri[:], in_=u_tri[:], compare_op=ALU.is_ge,
                fill=0.0, base=0, pattern=[[1, 128]], channel_multiplier=-1)
            # LT[l, j] = 1 iff j <= l (lower triangular incl diag; mask for diag block)
            lt_tri = pconst.tile([128, 128], BF16, tag="lt_tri")
            nc.gpsimd.memset(lt_tri[:], 1.0)
            nc.gpsimd.affine_select(   # keep where l - j >= 0
                out=lt_tri[:], in_=lt_tri[:], compare_op=ALU.is_ge,
                fill=0.0, base=0, pattern=[[-1, 128]], channel_multiplier=1)
            # L'[l', l] = 1 iff l' > l (strictly lower; cumsum release)
            lp_tri = pconst.tile([128, 128], F16, tag="lp_tri")
            nc.gpsimd.memset(lp_tri[:], 1.0)
            nc.gpsimd.affine_select(   # keep where l' - l > 0
                out=lp_tri[:], in_=lp_tri[:], compare_op=ALU.is_gt,
                fill=0.0, base=0, pattern=[[-1, 128]], channel_multiplier=1)
            ones_t = pconst.tile([128, 128], F16, tag="ones_t")
            nc.gpsimd.memset(ones_t[:], 1.0)

            qT = [pqt.tile([128, L], F16, tag=f"qT{i}", name=f"qT{i}") for i in range(NT)]
            v_sb = [pv.tile([128, D], BF16, tag=f"v{i}", name=f"v{i}") for i in range(NT)]
            attnT = [pat.tile([128, L], BF16, tag=f"at{i}", name=f"at{i}") for i in range(NT)]
            xt = pin.tile([128, NT, L], F16, tag="xt")
            wq = pin.tile([128, NT, D], F16, tag="wq")
            wv = pin.tile([128, NT, D], F16, tag="wv")
            wp_w = pin.tile([128, NT, D], BF16, tag="wpw")

            for _rep in range(reps):
                if _rep > 0:
                    tc.strict_bb_all_engine_barrier()

                # ---------------- input DMAs (spread across the 3 DMA-issuing queues) ----------------
                for k in range(NT):
                    nc.sync.dma_start(xt[:, k, :], xT_d[k])
                    nc.scalar.dma_start(wq[:, k, :], wqT[k])
                nc.scalar.dma_start(wp_w[:], wpT.rearrange("e p f -> p e f"))
                g_tiles = {}

                def load_gen(h):
                    g = pgen.tile([128, 2, GW], F16, tag="gen")
                    nc.gpsimd.dma_start(g[:], gen_d[h].rearrange("t p j -> p t j"))
                    g_tiles[h] = g

                load_gen(0)
                load_gen(1)
                for k in range(NT):
                    nc.gpsimd.dma_start(wv[:, k, :], wvT[k])
                load_gen(2)   # bufs=3: lands well before head 2

                # ---------------- GEMM job list (interleaved into head-loop slots) ------------
                def copy_ps(i, dst, src):
                    if i % 2 == 0:
                        nc.vector.tensor_copy(dst, src)
                    else:
                        nc.scalar.activation(dst, src, AF.Copy)

                def q_job(et):
                    for lc in range(2):
                        ps = psA.tile([128, 512], F32, tag="ps")
                        for k in range(NT):
                            nc.tensor.matmul(
                                ps[:], wq[:, k, et * 128:(et + 1) * 128],
                                xt[:, k, lc * 512:(lc + 1) * 512],
                                start=(k == 0), stop=(k == NT - 1))
                        copy_ps(lc, qT[et][:, lc * 512:(lc + 1) * 512], ps[:])

                def v_job(lt, ec):
                    ps = psA.tile([128, 512], F32, tag="ps")
                    for k in range(NT):
                        nc.tensor.matmul(
                            ps[:], xt[:, k, lt * 128:(lt + 1) * 128],
                            wv[:, k, ec * 512:(ec + 1) * 512],
                            start=(k == 0), stop=(k == NT - 1))
                    copy_ps(lt, v_sb[lt][:, ec * 512:(ec + 1) * 512], ps[:])

                def proj_job(lt):
                    osb = po.tile([128, D], F32, tag="osb")
                    for fc in range(2):
                        ps = psA.tile([128, 512], F32, tag="ps")
                        for e in range(NT):
                            nc.tensor.matmul(
                                ps[:], attnT[e][:, lt * 128:(lt + 1) * 128],
                                wp_w[:, e, fc * 512:(fc + 1) * 512],
                                start=(e == 0), stop=(e == NT - 1))
                        copy_ps(fc, osb[:, fc * 512:(fc + 1) * 512], ps[:])
                    (nc.scalar if lt % 2 else nc.sync).dma_start(out_d[lt], osb[:])

                # jobs consumed one-or-two per (h, k) slot. Tile deps are
                # emission-order-forward, so every producer must be EMITTED
                # before its first reader: v halves before the AV that reads
                # them (h0 AV-c0 at slot 3, AV-c1 at slot 7, e1 from slot 19),
                # qT[2h..2h+1] before head h starts (slots 8/16/24).
                jobs = []
                jobs += [lambda l=l: v_job(l, 0) for l in (0, 1, 2, 3)]
                jobs += [lambda e=e: q_job(e) for e in (2, 3)]
                jobs += [lambda l=l: v_job(l, 0) for l in (4, 5, 6, 7)]
                jobs += [lambda e=e: q_job(e) for e in (4, 5, 6, 7)]
                jobs += [lambda l=l: v_job(l, 1) for l in range(NT)]
                # slot id (0..31 = h*8+k) -> how many jobs to emit
                njobs = {s: (2 if s < 8 else 1) for s in range(14)}

                # ---------------- initial q tiles for head 0 ----------------
                q_job(0)
                q_job(1)

                def head_step(h, k, slot, W, C):
                    g = g_tiles[h]
                    s = win_start(k)
                    if k == 0:
                        # zero the never-written W regions AV reads (l < 128j)
                        for j in (1, 2, 3):
                            nc.gpsimd.memset(W[:, j, 0:128 * j], 0.0)
                        for j in (5, 6, 7):
                            nc.gpsimd.memset(W[:, j, 512:128 * j], 0.0)

                    for n in range(njobs.get(slot, 0)):
                        if jobs:
                            jobs.pop(0)()

                    # --- w_raw window GEMM (chunks 512/512/128, shared psA banks) ---
                    stg = pstg.tile([128, WW], F16, tag="stg")
                    for ci, (c0, cw) in enumerate(((0, 512), (512, 512), (1024, 128))):
                        wr = psA.tile([128, 512], F32, tag="ps")
                        for dd in range(2):
                            nc.tensor.matmul(
                                wr[:, 0:cw],
                                qT[2 * h + dd][:, k * 128:(k + 1) * 128],
                                g[:, dd, s + c0: s + c0 + cw],
                                start=(dd == 0), stop=(dd == 1))
                        if ci == 0 or (ci == 2 and k % 2 == 0):
                            nc.scalar.activation(stg[:, c0:c0 + cw], wr[:, 0:cw], AF.Copy)
                        else:
                            nc.vector.tensor_copy(stg[:, c0:c0 + cw], wr[:, 0:cw])
                    scr = pscr.tile([128, WW], F16, tag="scr")
                    nc.sync.dma_start(scr[:], stg[:])
                    # --- skew load: w2[p, i] = scr[p, off0 - p + i] ---
                    off0 = 1023 - 128 * k - s
                    w2 = pw2.tile([128, L], F16, tag="w2")
                    diag = AP(tensor=scr.tensor, offset=scr.offset + off0,
                              ap=[[WW - 1, 128], [1, L]])
                    nc.sync.dma_start(w2[:], diag)
                    # --- cumsum: C += U @ w2; chunk 1 unread until k=4 -> use F=U+L' ---
                    # skip_group_check: C stays a running PSUM accumulator across
                    # all k while the relu below reads it mid-group (fine on HW;
                    # the sim's group bookkeeping would reject both sides).
                    nc.tensor.matmul(C[:, 0:512], u_tri[:], w2[:, 0:512],
                                     start=(k == 0), stop=(k == NT - 1),
                                     skip_group_check=True)
                    c1_m = ones_t if k <= 3 else u_tri
                    nc.tensor.matmul(C[:, 512:1024], c1_m[:], w2[:, 512:1024],
                                     start=(k == 0), stop=(k == NT - 1),
                                     skip_group_check=True)
                    # --- relu copy + rowsum; diag block masked by tril ---
                    ncols = 128 * (k + 1)
                    wp_t = pwp.tile([128, L], BF16, tag="wp")
                    zrb = pz.tile([128, 1], F32, tag="zrb")
                    zri = pz.tile([128, 1], F32, tag="zri")
                    nc.scalar.activation(wp_t[:, k * 128:ncols], C[:, k * 128:ncols], AF.Relu)
                    nc.vector.tensor_mul(wp_t[:, k * 128:ncols], wp_t[:, k * 128:ncols], lt_tri[:])
                    nc.vector.tensor_reduce(zrb[:], wp_t[:, k * 128:ncols], axis=AX.X, op=ALU.add)
                    if k > 0:
                        zra = pz.tile([128, 1], F32, tag="zra")
                        nc.scalar.activation(wp_t[:, 0:k * 128], C[:, 0:k * 128], AF.Relu,
                                             accum_out=zra[:])
                        nc.vector.tensor_add(zrb[:], zra[:], zrb[:])
                    nc.vector.tensor_scalar_add(zrb[:], zrb[:], 1e-8)
                    nc.vector.reciprocal(zri[:], zrb[:])
                    # --- release C: += L' @ w2 (chunk 1 already has F for k<=3) ---
                    if k < NT - 1:
                        nc.tensor.matmul(C[:, 0:512], lp_tri[:], w2[:, 0:512],
                                         start=False, stop=False,
                                         skip_group_check=True)
                        if k > 3:
                            nc.tensor.matmul(C[:, 512:1024], lp_tri[:], w2[:, 512:1024],
                                             start=False, stop=False,
                                             skip_group_check=True)
                    # --- D = ident * (1/rowsum): transpose-with-normalize weights ---
                    Dm = pD.tile([128, 128], BF16, tag="D")
                    nc.vector.tensor_scalar_mul(Dm[:], ident[:], zri[:])
                    # --- transposes: W^T[j][:, k-block] = (wp_t[:, j-block])^T @ D ---
                    for gi, jp in enumerate(range(0, k + 1, 4)):
                        nj = min(4, k + 1 - jp)
                        tp = psT.tile([128, 512], F32, tag="tp")
                        for ji in range(nj):
                            j = jp + ji
                            nc.tensor.matmul(
                                tp[:, ji * 128:(ji + 1) * 128],
                                wp_t[:, j * 128:(j + 1) * 128], Dm[:],
                                start=True, stop=True)
                        dst = W[:, jp:jp + nj, k * 128:(k + 1) * 128]
                        srcv = tp[:, :nj * 128].rearrange("p (j l) -> p j l", j=nj)
                        if (k + gi) % 2 == 0:
                            nc.scalar.activation(dst, srcv, AF.Copy)
                        else:
                            nc.vector.tensor_copy(dst, srcv)
                    # --- AV for completed l-chunk ---
                    if k == 3 or k == 7:
                        c = k // 4
                        # sub-chunks (start, width, jmax): skip all-zero W regions
                        subs = ([(0, 512, 3)] if c == 0
                                else [(512, 256, 5), (768, 256, 7)])
                        for m in range(2):
                            av = psV.tile([128, 512], F32, tag="av")
                            for s0, sw, jmax in subs:
                                for j in range(jmax + 1):
                                    nc.tensor.matmul(
                                        av[:, s0 - c * 512: s0 - c * 512 + sw],
                                        v_sb[j][:, h * DH + m * 128: h * DH + (m + 1) * 128],
                                        W[:, j, s0:s0 + sw],
                                        start=(j == 0), stop=(j == jmax))
                            copy_ps(m, attnT[2 * h + m][:, c * 512:(c + 1) * 512], av[:])

                # ---------------- heads; proj l<512 overlaps head 3 tail ----------------
                slot = 0
                for h in range(H):
                    if h == 1:
                        load_gen(3)  # buf reused after head 0's last read
                    W = pW.tile([128, NT, L], BF16, tag="W")
                    C = psC.tile([128, L], F32, tag="C")
                    for k in range(NT):
                        head_step(h, k, slot, W, C)
                        slot += 1
                        if h == H - 1 and k >= 4:
                            proj_job(k - 4)   # lt 0..3: all heads' c0 AV done at h3 k3
                for lt in range(4, 8):
                    proj_job(lt)

    return nc


def make_in_maps(x, generator, Wq, Wv, Wproj):
    """Full inputs -> list of 8 per-core input dicts."""
    B = x.shape[0]
    bf16 = ml_dtypes.bfloat16
    wqT = np.ascontiguousarray(Wq.T.astype(np.float16)).reshape(NT, 128, D)
    wvT = np.ascontiguousarray(Wv.T.astype(np.float16)).reshape(NT, 128, D)
    wpT = np.ascontiguousarray(Wproj.T.astype(bf16)).reshape(NT, 128, D)
    gen = np.ascontiguousarray(generator.astype(np.float16)).reshape(H, 2, 128, GW)
    maps = []
    for b in range(B):
        xT = np.ascontiguousarray(x[b].T.astype(np.float16)).reshape(NT, 128, L)
        maps.append({"xT": xT, "wqT": wqT, "wvT": wvT, "gen": gen, "wpT": wpT})
    return maps


# ---------------------------------------------------------------------------
# post-pass: ISA instructions carry one wait slot; split extras onto NoOps


def split_excess_waits(nc, keep=1):
    n_split = 0
    for fn in nc.m.functions:
        for blk in fn.blocks:
            insts = list(blk.instructions)
            out = []
            changed = False
            for inst in insts:
                si = inst.sync_info
                waits = list(si.on_wait) if si is not None and si.on_wait else []
                if len(waits) > keep:
                    for j, w in enumerate(waits[:-keep]):
                        nop = mybir.InstNoOp(name=f"{inst.name}-ws{j}", ins=[], outs=[])
                        nop.engine = inst.engine
                        nop.sync_info = mybir.SyncInfo(on_wait=[w], on_update=[])
                        out.append(nop)
                        nc.register_instruction(nop, overwrite=True)
                    inst.sync_info = mybir.SyncInfo(
                        on_wait=waits[-keep:],
                        on_update=list(si.on_update) if si.on_update else [],
                    )
                    changed = True
                    n_split += 1
                out.append(inst)
            if changed:
                try:
                    blk.instructions = out
                except Exception:
                    blk.instructions.clear()
                    blk.instructions.extend(out)
    return n_split

# ---------------------------------------------------------------------------
class _SpmdRunner:
    def __init__(self, nc, n_cores):
        install_neuronx_cc_hook()
        self.nc = nc
        self.n_cores = n_cores
        partition_name = nc.partition_id_tensor.name if nc.partition_id_tensor else None
        in_names, out_names, out_avals = [], [], []
        for alloc in nc.m.functions[0].allocations:
            if not isinstance(alloc, mybir.MemoryLocationSet):
                continue
            name = alloc.memorylocations[0].name
            if alloc.kind == "ExternalInput":
                if name != partition_name:
                    in_names.append(name)
            elif alloc.kind == "ExternalOutput":
                out_names.append(name)
                out_avals.append(jax.core.ShapedArray(
                    tuple(alloc.tensor_shape), mybir.dt.np(alloc.dtype)))
        self.in_names, self.out_names, self.out_avals = in_names, out_names, out_avals
        n_params, n_outs = len(in_names), len(out_avals)
        all_in = in_names + out_names + ([partition_name] if partition_name else [])

        def _body(*args):
            operands = list(args)
            if partition_name is not None:
                operands.append(partition_id_tensor())
            return tuple(_bass_exec_p.bind(
                *operands, out_avals=tuple(out_avals), in_names=tuple(all_in),
                out_names=tuple(out_names), lowering_input_output_aliases=(),
                sim_require_finite=False, sim_require_nnan=False, nc=nc))

        devices = jax.devices()[:n_cores]
        assert len(devices) == n_cores, f"need {n_cores} neuron cores, have {len(jax.devices())}"
        self.mesh = Mesh(np.asarray(devices), ("core",))
        in_specs = (PartitionSpec("core"),) * (n_params + n_outs)
        out_specs = (PartitionSpec("core"),) * n_outs
        self.fn = jax.jit(shard_map(_body, mesh=self.mesh, in_specs=in_specs,
                                    out_specs=out_specs, check_rep=False),
                          keep_unused=True)
        self._dev_args = None

    def set_inputs(self, in_maps):
        n = self.n_cores
        args = [np.concatenate([np.asarray(in_maps[c][nm]) for c in range(n)], axis=0)
                for nm in self.in_names]
        for av in self.out_avals:
            args.append(np.zeros((n * av.shape[0], *av.shape[1:]), av.dtype))
        sharding = jax.sharding.NamedSharding(self.mesh, PartitionSpec("core"))
        self._dev_args = [jax.device_put(a, sharding) for a in args]

    def run(self):
        outs = self.fn(*self._dev_args)
        jax.block_until_ready(outs)
        return outs

    def outputs_np(self, outs):
        n = self.n_cores
        return [{nm: np.asarray(outs[i]).reshape(n, *self.out_avals[i].shape)[c]
                 for i, nm in enumerate(self.out_names)} for c in range(n)]


_CACHE = {}


def _get_runner(reps=1):
    if reps not in _CACHE:
        nc = bass.Bass(target_bir_lowering=False)
        build(nc, reps=reps)
        split_excess_waits(nc)
        _CACHE[reps] = _SpmdRunner(nc, 8)
    return _CACHE[reps]


def kernel(x, generator, Wq, Wv, Wproj):
    x = np.asarray(x); generator = np.asarray(generator)
    in_maps = make_in_maps(x, generator, np.asarray(Wq), np.asarray(Wv), np.asarray(Wproj))
    runner = _get_runner()
    runner.set_inputs(in_maps)
    outs = runner.outputs_np(runner.run())
    return np.stack([outs[b]["out"].reshape(L, D) for b in range(x.shape[0])]).astype(np.float32)
